# revision 1
# baseline (speedup 1.0000x reference)
"""Trainium2 Bass kernel for nn_EqStftSnsePBC (STFT -> per-tap nonlinear PBC -> ISTFT).

Strategy (8 NeuronCores, pure data parallelism over STFT frames):
  host:   frame the signal (hop 216, n_fft 256) into [stream, freq_in_frame, frame]
          arrays, shard 4632 (zero-padded) frames as 579 per core; build DFT/IDFT
          lhsT matrices and the 256x256 wrap-around Toeplitz correlation matrix G
          (negated, launch power P and the 1/256 IDFT scale folded in).
  device: computes only the perturbation V = IDFT((j*P*phi) .* X), all in bf16
          (f32 PSUM accumulation):
            X  = DFT(frames)        (matmuls, K=256 contracted in 2x128 chunks)
            I  = sum_modes |X|^2    (DVE)
            na, nb = G @ I          (matmuls; = -P*phi_r, -P*phi_i)
            U  = nb.*X + j(...)     (DVE + GPSIMD elementwise)
            V  = IDFT(U)            (matmuls)
  host:   yf = frames + V (exact passthrough of the linear term), overlap-add,
          divide by coverage count, trim, restack.

Measured on trn2 (8 cores): ~88-91 us HW exec, rel err ~8.8e-3 vs fp32 reference.
"""

import os
import sys

for _p in ("/opt/trn_rl_repo",):
    if os.path.isdir(_p) and _p not in sys.path:
        sys.path.append(_p)

import numpy as np
try:
    import ml_dtypes
    _BF16 = np.dtype(ml_dtypes.bfloat16)
except Exception:
    _BF16 = None

# ---- problem geometry (hardcoded) ----
MTAPS = 41
PAD = MTAPS // 2  # 20
NFFT = 256
HOP = 216
B = 2
NM = 2
L = 999688
STEPS = 4628            # (L - NFFT) // HOP + 1
NCORES = 8
NH = 579                # frames per core
FTOT = NCORES * NH      # 4632 >= STEPS (4 trailing fake frames, ignored on host)
LOUT = L - 2 * PAD * 2  # L - overlaps = 999648
NBLOCKS = ((0, 290), (289, 290))   # (col offset, width); even widths >= 256 (fp32r ISA); 1-col overlap is benign
NSTREAMS = B * NM * 2   # (b, mode, re/im) -> 8

_PROG = None            # (nc, input_names) cache; compile once per process
LAST_EXEC_NS = None
LAST_RESULTS = None


def _build_const_matrices(h_real, h_imag, task_info):
    """DFT/IDFT lhsT matrices and per-batch P-scaled correlation matrices."""
    n = np.arange(NFFT)
    ang = 2.0 * np.pi * np.outer(n, n) / NFFT
    c, s = np.cos(ang), np.sin(ang)
    # forward lhsT[n, f] so that lhsT.T @ frames = W @ frames, W = exp(-2i pi f n / N)
    # stages: 0 fwd_r, 1 fwd_i, 2 fwd_minus_i, 3 inv_r, 4 inv_i, 5 inv_minus_i
    wmat = np.empty((12, 128, NFFT), np.float32)
    stages = [c, -s, s, c / NFFT, s / NFFT, -s / NFFT]
    for st, mat in enumerate(stages):
        wmat[st * 2 + 0] = mat[0:128, :].astype(np.float32)
        wmat[st * 2 + 1] = mat[128:256, :].astype(np.float32)

    # G[p', m] = sum of h[p-m] over p in [0,296) with (p-20) mod 256 == p'
    def toep(h):
        G = np.zeros((NFFT, NFFT), np.float64)
        for p in range(NFFT + 2 * PAD):
            pp = (p - PAD) % NFFT
            lo, hi = max(0, p - (MTAPS - 1)), min(NFFT - 1, p)
            if lo <= hi:
                ms = np.arange(lo, hi + 1)
                G[pp, ms] += h[p - ms]
        return G

    Gr, Gi = toep(np.asarray(h_real, np.float64)), toep(np.asarray(h_imag, np.float64))
    P = (10.0 ** (np.asarray(task_info, np.float64)[:, 0] / 10.0) / NM)
    # negated so the device computes na = -P*phi_r, nb = -P*phi_i directly
    gmat = np.empty((B, 4, 128, NFFT), np.float32)
    for b in range(B):
        for kc in range(2):
            gmat[b, 0 * 2 + kc] = (-P[b] * Gr[kc * 128:(kc + 1) * 128, :]).astype(np.float32)
            gmat[b, 1 * 2 + kc] = (-P[b] * Gi[kc * 128:(kc + 1) * 128, :]).astype(np.float32)
    # packed device layouts: [128, T*256] with T-major columns, all bf16
    wall_packed = np.ascontiguousarray(
        wmat.transpose(1, 0, 2).reshape(128, 12 * NFFT)).astype(_BF16)
    gall_packed = np.ascontiguousarray(
        gmat.reshape(B * 4, 128, NFFT).transpose(1, 0, 2).reshape(128, B * 4 * NFFT)
    ).astype(_BF16)
    return wmat, gmat, wall_packed, gall_packed


def _frame_inputs(x_real, x_imag):
    """-> xf [B, NM, 2, NFFT, FTOT] fp32: xf[b,m,ri,n,j] = x[b, HOP*j + n, m]."""
    need = HOP * (FTOT - 1) + NFFT
    xf = np.empty((B, NM, 2, NFFT, FTOT), np.float32)
    for ri, x in enumerate((x_real, x_imag)):
        xt = np.ascontiguousarray(np.asarray(x).transpose(0, 2, 1))  # [B, NM, L]
        xp = np.zeros((B, NM, need), np.float32)
        xp[:, :, :L] = xt
        sw = np.lib.stride_tricks.as_strided(
            xp,
            shape=(B, NM, FTOT, NFFT),
            strides=(xp.strides[0], xp.strides[1], HOP * 4, 4),
        )
        xf[:, :, ri] = sw.transpose(0, 1, 3, 2)
    return xf


def _overlap_add(yf):
    """yf [B, NM, 2, NFFT, FTOT] -> y [B, NM, 2, L] (OLA / coverage)."""
    y = np.zeros((B, NM, 2, STEPS, HOP), np.float32)
    body = yf[:, :, :, :HOP, :STEPS].transpose(0, 1, 2, 4, 3)  # [.., j, 216]
    y[:] = body
    tail = yf[:, :, :, HOP:, :STEPS - 1].transpose(0, 1, 2, 4, 3)  # [.., j, 40]
    y[:, :, :, 1:, :NFFT - HOP] += tail
    y = y.reshape(B, NM, 2, STEPS * HOP)
    yfull = np.empty((B, NM, 2, L), np.float32)
    yfull[:, :, :, :STEPS * HOP] = y
    yfull[:, :, :, STEPS * HOP:] = yf[:, :, :, HOP:, STEPS - 1]  # final tail, coverage 1
    # coverage: 2 on [216(j+1), 216(j+1)+40) for j=0..steps-2, else 1
    t = np.arange(L)
    wsum = np.ones(L, np.float32)
    wsum[(t >= HOP) & (t < STEPS * HOP) & (t % HOP < NFFT - HOP)] = 2.0
    yfull /= wsum
    return yfull


def _build_program():
    import concourse.bass as bass
    import concourse.tile as tile
    from concourse import bacc, mybir
    from contextlib import ExitStack

    f32 = mybir.dt.float32
    f32r = mybir.dt.float32r
    bf16 = mybir.dt.bfloat16
    MULT = mybir.AluOpType.mult
    ADD = mybir.AluOpType.add
    SUB = mybir.AluOpType.subtract

    nc = bacc.Bacc(None, target_bir_lowering=False, debug=False)
    xf_d = nc.dram_tensor("xf", [NSTREAMS, NFFT, NH], bf16, kind="ExternalInput").ap()
    wall_d = nc.dram_tensor("wall", [128, 12 * NFFT], bf16, kind="ExternalInput").ap()
    gmb_d = nc.dram_tensor("gmatb", [128, B * 4 * NFFT], bf16, kind="ExternalInput").ap()
    vf_d = nc.dram_tensor("vf", [NSTREAMS, NFFT, NH], bf16, kind="ExternalOutput").ap()

    FWD_R, FWD_I, FWD_MI, INV_R, INV_I, INV_MI = range(6)
    FFT_TERMS = {0: ((FWD_R, 0), (FWD_MI, 1)),   # Xr = Wr xr - Wi xi
                 1: ((FWD_R, 1), (FWD_I, 0))}    # Xi = Wr xi + Wi xr
    IFFT_TERMS = {0: ((INV_R, 0), (INV_MI, 1)),  # Vr = iWr Ur - iWi Ui
                  1: ((INV_R, 1), (INV_I, 0))}   # Vi = iWr Ui + iWi Ur

    BLOCKS = [(b, j0, NB) for b in range(B) for (j0, NB) in NBLOCKS]

    with tile.TileContext(nc) as tc:
        with ExitStack() as ctx:
            consts = ctx.enter_context(tc.tile_pool(name="consts", bufs=1))
            NBUFS = int(os.environ.get("K_BUFS", "3"))
            xin = ctx.enter_context(tc.tile_pool(name="xin", bufs=NBUFS))
            xcp = ctx.enter_context(tc.tile_pool(name="xcp", bufs=NBUFS))
            work = ctx.enter_context(tc.tile_pool(name="work", bufs=NBUFS))
            usb_p = ctx.enter_context(tc.tile_pool(name="usb", bufs=NBUFS))
            osb_p = ctx.enter_context(tc.tile_pool(name="osb", bufs=NBUFS))
            ps_x = ctx.enter_context(tc.tile_pool(name="psx", bufs=4, space="PSUM"))
            ps_phi = ctx.enter_context(tc.tile_pool(name="psphi", bufs=2, space="PSUM"))
            ps_v = ctx.enter_context(tc.tile_pool(name="psv", bufs=2, space="PSUM"))

            # constants, consolidated into 3 DMAs (startup latency)
            wall = consts.tile([128, 12 * NFFT], bf16, tag="wall")
            nc.sync.dma_start(wall[:], wall_d[:])
            gall = consts.tile([128, B * 4 * NFFT], bf16, tag="gall")
            nc.scalar.dma_start(gall[:], gmb_d[:])
            wsb = {}
            for st in range(6):
                for kc in range(2):
                    for mh in range(2):
                        off = (st * 2 + kc) * NFFT + mh * 128
                        wsb[(st, kc, mh)] = wall[:, off:off + 128]
            gsb = {}
            for b in range(B):
                for t in range(4):
                    for mh in range(2):
                        off = (b * 4 + t) * NFFT + mh * 128
                        gsb[(b, t, mh)] = gall[:, off:off + 128]

            def sidx(b, m, ri):
                return b * 4 + m * 2 + ri

            state = {}

            def emit_load_fft(t):
                """DMA frames in, FFT (f32r, m-paired weights), cast X to bf16,
                and intensity — fills state[t]."""
                b, j0, NB = BLOCKS[t]
                xsb = {}
                for m in range(NM):
                    for ri in range(2):
                        for kc in range(2):
                            tl = xin.tile([128, NB], bf16, tag=f"x{m}{ri}{kc}",
                                          name=f"x{t}_{m}{ri}{kc}")
                            nc.sync.dma_start(
                                tl[:],
                                xf_d[sidx(b, m, ri), kc * 128:(kc + 1) * 128,
                                     j0:j0 + NB],
                            )
                            xsb[(m, ri, kc)] = tl
                Xsb = {}
                for ri_o in range(2):
                    for mh in range(2):
                        xps = [ps_x.tile([128, NB], f32, tag="xps",
                                         name=f"xps{t}_{ri_o}{mh}{_m}") for _m in range(NM)]
                        seq = [(st, src, kc)
                               for (st, src) in FFT_TERMS[ri_o] for kc in range(2)]
                        if PAIRED:
                            for i, (st, src, kc) in enumerate(seq):
                                for m in range(NM):
                                    nc.tensor.matmul(
                                        xps[m][:], wsb[(st, kc, mh)],
                                        xsb[(m, src, kc)][:],
                                        start=(i == 0), stop=(i == len(seq) - 1),
                                    )
                        else:
                            for m in range(NM):
                                for i, (st, src, kc) in enumerate(seq):
                                    nc.tensor.matmul(
                                        xps[m][:], wsb[(st, kc, mh)],
                                        xsb[(m, src, kc)][:],
                                        start=(i == 0), stop=(i == len(seq) - 1),
                                    )
                        xs = xcp.tile([128, 2 * NB], bf16, tag=f"X{ri_o}{mh}",
                                      name=f"X{t}_{ri_o}{mh}")
                        for m in range(NM):
                            dst = xs[:, m * NB:(m + 1) * NB]
                            if (m + ri_o) % 2 == 0:
                                nc.vector.tensor_copy(dst, xps[m][:])
                            else:
                                nc.scalar.copy(dst, xps[m][:])
                        Xsb[(ri_o, mh)] = xs
                isb = {}
                for mh in range(2):
                    it = work.tile([128, NB], bf16, tag=f"i{mh}", name=f"i{t}_{mh}")
                    t0 = work.tile([128, 2 * NB], bf16, tag=f"sqa{mh}", name=f"sqa{t}_{mh}")
                    t1 = work.tile([128, 2 * NB], bf16, tag=f"sqb{mh}", name=f"sqb{t}_{mh}")
                    nc.vector.tensor_tensor(t0[:], Xsb[(0, mh)][:], Xsb[(0, mh)][:], MULT)
                    nc.vector.tensor_tensor(t1[:], Xsb[(1, mh)][:], Xsb[(1, mh)][:], MULT)
                    nc.vector.tensor_tensor(t0[:], t0[:], t1[:], ADD)
                    nc.vector.tensor_tensor(it[:], t0[:, 0:NB], t0[:, NB:2 * NB], ADD)
                    isb[mh] = it
                state[t] = {"xsb": xsb, "Xsb": Xsb, "isb": isb}

            def emit_corr_u(t):
                """corr matmuls, phi copies, U elementwise for block t."""
                b, j0, NB = BLOCKS[t]
                st_ = state[t]
                nab = {}
                for mh in range(2):
                    for ri in range(2):
                        pp = ps_phi.tile([128, NB], f32, tag="phps",
                                         name=f"ph{t}_{ri}{mh}")
                        for kc in range(2):
                            nc.tensor.matmul(
                                pp[:], gsb[(b, ri * 2 + kc, mh)], st_["isb"][kc][:],
                                start=(kc == 0), stop=(kc == 1),
                            )
                        ab = work.tile([128, 2 * NB], bf16, tag=f"ab{ri}{mh}",
                                       name=f"ab{t}_{ri}{mh}")
                        nc.scalar.copy(ab[:, 0:NB], pp[:])
                        nc.scalar.copy(ab[:, NB:2 * NB], pp[:])
                        nab[(ri, mh)] = ab
                usb = {}
                for mh in range(2):
                    na, nb_ = nab[(0, mh)], nab[(1, mh)]
                    Xr, Xi = st_["Xsb"][(0, mh)], st_["Xsb"][(1, mh)]
                    t0 = work.tile([128, 2 * NB], bf16, tag=f"ut0{mh}", name=f"ut0{t}_{mh}")
                    t1 = work.tile([128, 2 * NB], bf16, tag=f"ut1{mh}", name=f"ut1{t}_{mh}")
                    t2 = work.tile([128, 2 * NB], bf16, tag=f"ut2{mh}", name=f"ut2{t}_{mh}")
                    t3 = work.tile([128, 2 * NB], bf16, tag=f"ut3{mh}", name=f"ut3{t}_{mh}")
                    ur = usb_p.tile([128, 2 * NB], bf16, tag=f"ur{mh}", name=f"ur{t}_{mh}")
                    ui = usb_p.tile([128, 2 * NB], bf16, tag=f"ui{mh}", name=f"ui{t}_{mh}")
                    # muls split GPS/DVE so the two operands of each add run in parallel
                    nc.gpsimd.tensor_tensor(t0[:], nb_[:], Xr[:], MULT)
                    nc.vector.tensor_tensor(t1[:], na[:], Xi[:], MULT)
                    nc.gpsimd.tensor_tensor(t2[:], nb_[:], Xi[:], MULT)
                    nc.vector.tensor_tensor(t3[:], na[:], Xr[:], MULT)
                    nc.vector.tensor_tensor(ur[:], t0[:], t1[:], ADD)
                    nc.vector.tensor_tensor(ui[:], t2[:], t3[:], SUB)
                    usb[(0, mh)] = ur
                    usb[(1, mh)] = ui
                st_["usb"] = usb

            def emit_ifft(t):
                b, j0, NB = BLOCKS[t]
                usb = state[t]["usb"]
                for ri_o in range(2):
                    for nh in range(2):
                        vps = [ps_v.tile([128, NB], f32, tag="vps",
                                         name=f"vps{t}_{ri_o}{nh}{_m}") for _m in range(NM)]
                        seq = [(st, src, kc)
                               for kc in (1, 0) for (st, src) in IFFT_TERMS[ri_o]]
                        for i, (st, src, kc) in enumerate(seq):
                            for m in range(NM):
                                nc.tensor.matmul(
                                    vps[m][:], wsb[(st, kc, nh)],
                                    usb[(src, kc)][:, m * NB:(m + 1) * NB],
                                    start=(i == 0), stop=(i == len(seq) - 1),
                                )
                        for m in range(NM):
                            ob = osb_p.tile([128, NB], bf16, tag=f"o{m}{ri_o}{nh}",
                                            name=f"o{t}_{m}{ri_o}{nh}")
                            nc.scalar.copy(ob[:], vps[m][:])
                            nc.scalar.dma_start(
                                vf_d[sidx(b, m, ri_o), nh * 128:(nh + 1) * 128,
                                     j0:j0 + NB],
                                ob[:],
                            )
                del state[t]

            # software pipeline: corr/U of block t overlaps FFT of block t+1
            PIPELINE = os.environ.get("K_PIPELINE", "1") == "1"
            PAIRED = os.environ.get("K_PAIR", "1") == "1"
            if PIPELINE:
                emit_load_fft(0)
                for t in range(len(BLOCKS)):
                    emit_corr_u(t)
                    if t + 1 < len(BLOCKS):
                        emit_load_fft(t + 1)
                    emit_ifft(t)
            else:
                for t in range(len(BLOCKS)):
                    emit_load_fft(t)
                    emit_corr_u(t)
                    emit_ifft(t)

    nc.compile()
    return nc


def _run_device(xf, wall_packed, gall_packed, trace=False):
    """xf [B,NM,2,NFFT,FTOT] -> vf same shape, via 8-core SPMD bass kernel."""
    global _PROG, LAST_EXEC_NS, LAST_RESULTS
    from concourse.bass_utils import run_bass_kernel_spmd

    if _PROG is None:
        _PROG = _build_program()
    nc = _PROG

    xfs = xf.reshape(NSTREAMS, NFFT, FTOT).astype(_BF16)
    in_maps = []
    for k in range(NCORES):
        in_maps.append({
            "xf": np.ascontiguousarray(xfs[:, :, k * NH:(k + 1) * NH]),
            "wall": wall_packed,
            "gmatb": gall_packed,
        })
    kwargs = {}
    if trace:
        kwargs["trace"] = True
    res = run_bass_kernel_spmd(nc, in_maps, list(range(NCORES)), **kwargs)
    LAST_EXEC_NS = res.exec_time_ns
    LAST_RESULTS = res
    vf = np.empty((NSTREAMS, NFFT, FTOT), np.float32)
    for k in range(NCORES):
        vf[:, :, k * NH:(k + 1) * NH] = res.results[k]["vf"].astype(np.float32)
    return vf.reshape(B, NM, 2, NFFT, FTOT)


def _emulate_device(xf, wmat, gmat):
    """Numpy mirror of the device program: returns V = IFFT(j P phi * X)."""
    W = {st: np.concatenate([wmat[st * 2], wmat[st * 2 + 1]], 0) for st in range(6)}
    vf = np.empty_like(xf)
    for b in range(B):
        G = {ri: np.concatenate([gmat[b, ri * 2], gmat[b, ri * 2 + 1]], 0) for ri in range(2)}
        Xr = np.einsum('nf,mnj->mfj', W[0], xf[b, :, 0]) + np.einsum('nf,mnj->mfj', W[2], xf[b, :, 1])
        Xi = np.einsum('nf,mnj->mfj', W[0], xf[b, :, 1]) + np.einsum('nf,mnj->mfj', W[1], xf[b, :, 0])
        I = (Xr * Xr + Xi * Xi).sum(axis=0)
        na = G[0].T @ I    # = -P*phi_r
        nb = G[1].T @ I    # = -P*phi_i
        Ur, Ui = nb * Xr + na * Xi, nb * Xi - na * Xr
        vf[b, :, 0] = np.einsum('fn,mfj->mnj', W[3], Ur) + np.einsum('fn,mfj->mnj', W[5], Ui)
        vf[b, :, 1] = np.einsum('fn,mfj->mnj', W[3], Ui) + np.einsum('fn,mfj->mnj', W[4], Ur)
    return vf


def kernel(x_real, x_imag, task_info, h_real, h_imag, _emulate=False, _trace=False):
    x_real = np.asarray(x_real, np.float32)
    x_imag = np.asarray(x_imag, np.float32)
    wmat, gmat, wall_packed, gall_packed = _build_const_matrices(h_real, h_imag, task_info)
    xf = _frame_inputs(x_real, x_imag)
    if _emulate:
        vf = _emulate_device(xf, wmat, gmat)
    else:
        vf = _run_device(xf, wall_packed, gall_packed, trace=_trace)
    yf = xf + vf                              # exact passthrough + device correction
    y = _overlap_add(yf)                      # [B, NM, 2, L]
    y = y[:, :, :, PAD:L - PAD]               # trim overlaps//2 each side
    return np.ascontiguousarray(y.transpose(0, 3, 1, 2))  # [B, LOUT, NM, 2]



# revision 5
# speedup vs baseline: 1.3269x; 1.3269x over previous
"""Trainium2 Bass kernel for nn_EqStftSnsePBC (STFT -> per-tap nonlinear PBC -> ISTFT).

Strategy (8 NeuronCores, data parallel over STFT frames):
  host:   frame the signal (hop 216, n_fft 256), split each frame into even/odd
          time samples (radix-2 DIT), pack per-block DRAM buffers that mirror
          the SBUF tile layout exactly (one contiguous DMA per block each way).
  device: per block of NB=116 frames (modes packed along columns, 2NB=232):
            E  = W128 @ xe, F = (diag(w) W128) @ xo      (8 matmuls, K=128)
            X_lo = E + F, X_hi = E - F                   (DVE butterfly == PSUM copy)
            I  = sum_modes |X|^2                         (ACT square + DVE folds)
            phi = G @ I  (circulant corr, -P folded)     (8 matmuls)
            U  = j*P*phi .* X                            (DVE elementwise)
            v  = IDFT256 @ U  (dense, 2 row-chunks)      (16 matmuls)
          all bf16 with f32 PSUM accumulation.
  host:   yf = frames + v (exact passthrough of linear term), overlap-add,
          divide by coverage, trim, restack.
"""

import os
import sys

for _p in ("/opt/trn_rl_repo",):
    if os.path.isdir(_p) and _p not in sys.path:
        sys.path.append(_p)

import numpy as np
try:
    import ml_dtypes
    _BF16 = np.dtype(ml_dtypes.bfloat16)
except Exception:
    _BF16 = None

# ---- problem geometry (hardcoded) ----
MTAPS = 41
PAD = MTAPS // 2  # 20
NFFT = 256
HOP = 216
B = 2
NM = 2
L = 999688
STEPS = 4628            # (L - NFFT) // HOP + 1
NCORES = 8
NB = 116                # frames per block
NQ = 5                  # blocks per (core, b)
NH = NQ * NB            # 580 frames per core per b
FTOT = NCORES * NH      # 4640 >= STEPS (12 trailing fake frames, ignored on host)
NBLK = B * NQ           # 10 blocks per core
W2 = 2 * NB             # 232  (modes packed)
W4 = 4 * NB             # 464

_PROG = None
LAST_EXEC_NS = None
LAST_RESULTS = None


def _build_const_matrices(h_real, h_imag, task_info):
    """All lhsT constant matrices, bf16-packed for the device.

    wall [128, 18*128]: radix-2 FFT mats (ME, MF) and dense IDFT blocks,
      each as (Mr, Mi, -Mi) triples of lhsT = M.T.
    gall [128, 16*128]: correlation lhsT blocks per (b, ri, half, kc),
      scaled by -P[b].
    """
    n = np.arange(128)
    W128 = np.exp(-2j * np.pi * np.outer(n, n) / 128.0)        # [k, n]
    w = np.exp(-2j * np.pi * np.arange(128) / 256.0)           # twiddles
    ME = W128                                                   # E = ME @ xe
    MF = w[:, None] * W128                                      # F = MF @ xo
    t = np.arange(NFFT)
    IDFT = np.exp(2j * np.pi * np.outer(t, t) / NFFT) / NFFT    # [t, k]

    mats = [ME, MF]
    for tc in range(2):
        for kc in range(2):
            mats.append(IDFT[tc * 128:(tc + 1) * 128, kc * 128:(kc + 1) * 128])
    # -> 6 complex matrices -> 18 real lhsT blocks (Mr, Mi, -Mi each)
    wall = np.empty((18, 128, 128), np.float32)
    for i, M in enumerate(mats):
        lr, li = M.T.real, M.T.imag    # lhsT[n, k] = M[k, n]
        wall[3 * i + 0] = lr
        wall[3 * i + 1] = li
        wall[3 * i + 2] = -li

    # G[p', m] circulant correlation matrix (same as reference semantics)
    def toep(h):
        G = np.zeros((NFFT, NFFT), np.float64)
        for p in range(NFFT + 2 * PAD):
            pp = (p - PAD) % NFFT
            lo, hi = max(0, p - (MTAPS - 1)), min(NFFT - 1, p)
            if lo <= hi:
                ms = np.arange(lo, hi + 1)
                G[pp, ms] += h[p - ms]
        return G

    Gr = toep(np.asarray(h_real, np.float64))
    Gi = toep(np.asarray(h_imag, np.float64))
    P = 10.0 ** (np.asarray(task_info, np.float64)[:, 0] / 10.0) / NM
    # toep's G is [source_freq, output_freq]: phi = G.T @ I, so the lhsT
    # block for output-half `half`, input-chunk `kc` is G[kc rows, half cols].
    gall = np.empty((B, 2, 2, 2, 128, 128), np.float32)  # [b, ri, half, kc, n, k]
    for b in range(B):
        for ri, G in enumerate((Gr, Gi)):
            for half in range(2):
                for kc in range(2):
                    blk = G[kc * 128:(kc + 1) * 128, half * 128:(half + 1) * 128]
                    gall[b, ri, half, kc] = (-P[b] * blk)
    wall_p = np.ascontiguousarray(
        wall.transpose(1, 0, 2).reshape(128, 18 * 128)).astype(_BF16)
    gall_p = np.ascontiguousarray(
        gall.reshape(16, 128, 128).transpose(1, 0, 2).reshape(128, 16 * 128)
    ).astype(_BF16)
    return wall, gall, wall_p, gall_p


def _frames_view(x_real, x_imag):
    """-> F [B, NM, 2, FTOT, NFFT] float32 frames (zero-padded past L)."""
    need = HOP * (FTOT - 1) + NFFT
    F = np.empty((B, NM, 2, FTOT, NFFT), np.float32)
    for ri, x in enumerate((x_real, x_imag)):
        xt = np.ascontiguousarray(np.asarray(x, np.float32).transpose(0, 2, 1))
        xp = np.zeros((B, NM, need), np.float32)
        xp[:, :, :L] = xt
        sw = np.lib.stride_tricks.as_strided(
            xp, shape=(B, NM, FTOT, NFFT),
            strides=(xp.strides[0], xp.strides[1], HOP * 4, 4))
        F[:, :, ri] = sw
    return F


def _pack_inputs(F):
    """F [B,NM,2,FTOT,256] -> per-core xin [NCORES][NBLK,128,4,NM,NB] bf16.

    s index = eo*2 + ri: 0=(even,re) 1=(even,im) 2=(odd,re) 3=(odd,im).
    """
    xe = F[..., 0::2]   # [B, NM, 2, FTOT, 128]
    xo = F[..., 1::2]
    arr = np.stack([xe, xo], axis=0)  # [eo, B, NM, ri, FTOT, 128]
    r1 = arr.reshape(2, B, NM, 2, NCORES, NQ, NB, 128)
    # -> [k, b, q, n, eo, ri, m, j]
    out = r1.transpose(4, 1, 5, 7, 0, 3, 2, 6)
    out = np.ascontiguousarray(out).astype(_BF16)
    return out.reshape(NCORES, NBLK, 128, 4, NM, NB)


def _unpack_outputs(vouts):
    """vouts [NCORES][NBLK,128,2,2,NM,NB] -> v frames [B,NM,2,FTOT,256] f32."""
    va = np.stack([v.astype(np.float32) for v in vouts], axis=0)
    # dims [k, b, q, n, tc, ri, m, j] -> [b, m, ri, k, q, j, tc, n]
    va = va.reshape(NCORES, B, NQ, 128, 2, 2, NM, NB)
    vfr = va.transpose(1, 6, 5, 0, 2, 7, 4, 3).reshape(B, NM, 2, FTOT, NFFT)
    return vfr


def _overlap_add(yf):
    """yf [B, NM, 2, NFFT, FTOT] -> y [B, NM, 2, L] (OLA / coverage)."""
    y = np.zeros((B, NM, 2, STEPS, HOP), np.float32)
    body = yf[:, :, :, :HOP, :STEPS].transpose(0, 1, 2, 4, 3)
    y[:] = body
    tail = yf[:, :, :, HOP:, :STEPS - 1].transpose(0, 1, 2, 4, 3)
    y[:, :, :, 1:, :NFFT - HOP] += tail
    y = y.reshape(B, NM, 2, STEPS * HOP)
    yfull = np.empty((B, NM, 2, L), np.float32)
    yfull[:, :, :, :STEPS * HOP] = y
    yfull[:, :, :, STEPS * HOP:] = yf[:, :, :, HOP:HOP + (L - STEPS * HOP), STEPS - 1]
    t = np.arange(L)
    wsum = np.ones(L, np.float32)
    wsum[(t >= HOP) & (t < STEPS * HOP) & (t % HOP < NFFT - HOP)] = 2.0
    yfull /= wsum
    return yfull


def _build_program():
    import concourse.bass as bass
    import concourse.tile as tile
    from concourse import bacc, mybir
    from contextlib import ExitStack

    f32 = mybir.dt.float32
    bf16 = mybir.dt.bfloat16
    MULT = mybir.AluOpType.mult
    ADD = mybir.AluOpType.add
    SUB = mybir.AluOpType.subtract
    SQUARE = mybir.ActivationFunctionType.Square

    nc = bacc.Bacc(None, target_bir_lowering=False, debug=False)
    xin_d = nc.dram_tensor("xin", [NBLK, 128, 4, NM, NB], bf16,
                           kind="ExternalInput").ap()
    wall_d = nc.dram_tensor("wall", [128, 18 * 128], bf16,
                            kind="ExternalInput").ap()
    gall_d = nc.dram_tensor("gall", [128, 16 * 128], bf16,
                            kind="ExternalInput").ap()
    vout_d = nc.dram_tensor("vout", [NBLK, 128, 2, 2, NM, NB], bf16,
                            kind="ExternalOutput").ap()

    # wall block index: (mat, part) mat in [ME, MF, I00, I01, I10, I11],
    # part in [r, i, negi]
    def wslice(wall_sb, mat, part):
        off = (mat * 3 + part) * 128
        return wall_sb[:, off:off + 128]

    def gslice(gall_sb, b, ri, half, kc):
        off = (((b * 2 + ri) * 2 + half) * 2 + kc) * 128
        return gall_sb[:, off:off + 128]

    with tile.TileContext(nc) as tc:
        with ExitStack() as ctx:
            consts = ctx.enter_context(tc.tile_pool(name="consts", bufs=1))
            xin_p = ctx.enter_context(tc.tile_pool(name="xin", bufs=3))
            xsb_p = ctx.enter_context(tc.tile_pool(name="xsb", bufs=3))
            isb_p = ctx.enter_context(tc.tile_pool(name="isb", bufs=3))
            dup_p = ctx.enter_context(tc.tile_pool(name="dup", bufs=2))
            usb_p = ctx.enter_context(tc.tile_pool(name="usb", bufs=2))
            osb_p = ctx.enter_context(tc.tile_pool(name="osb", bufs=2))
            ps_fft = ctx.enter_context(tc.tile_pool(name="psf", bufs=2, space="PSUM"))
            ps_cor = ctx.enter_context(tc.tile_pool(name="psc", bufs=2, space="PSUM"))
            ps_ift = ctx.enter_context(tc.tile_pool(name="psv", bufs=1, space="PSUM"))

            wall = consts.tile([128, 18 * 128], bf16, tag="wall")
            nc.scalar.dma_start(wall[:], wall_d[:])
            gall = consts.tile([128, 16 * 128], bf16, tag="gall")
            nc.scalar.dma_start(gall[:], gall_d[:])

            state = {}

            def eA(t):
                """input DMA + FFT matmuls + butterfly combine -> X in SBUF."""
                xin = xin_p.tile([128, 4, NM, NB], bf16, tag="xin", name=f"xin{t}")
                nc.sync.dma_start(xin[:], xin_d[t])
                # E/F psum tiles: [:, 0] = real, [:, 1] = imag
                E = ps_fft.tile([128, 2, W2], f32, tag="E", name=f"E{t}")
                Fp = ps_fft.tile([128, 2, W2], f32, tag="F", name=f"F{t}")
                xer, xei = xin[:, 0], xin[:, 1]
                xor_, xoi = xin[:, 2], xin[:, 3]
                for (ps, mat, ur, ui) in ((E, 0, xer, xei), (Fp, 1, xor_, xoi)):
                    mr = wslice(wall, mat, 0)
                    mi = wslice(wall, mat, 1)
                    mni = wslice(wall, mat, 2)
                    nc.tensor.matmul(ps[:, 0], mr, ur, start=True, stop=False)
                    nc.tensor.matmul(ps[:, 0], mni, ui, start=False, stop=True)
                    nc.tensor.matmul(ps[:, 1], mr, ui, start=True, stop=False)
                    nc.tensor.matmul(ps[:, 1], mi, ur, start=False, stop=True)
                # butterfly: X_lo = E + F, X_hi = E - F  (to SBUF bf16).
                # TT can read only one PSUM operand, so stage F through SBUF.
                Fs = xsb_p.tile([128, 2, NM, NB], bf16, tag="Fs", name=f"Fs{t}")
                nc.scalar.copy(Fs[:], Fp[:])
                Xlo = xsb_p.tile([128, 2, NM, NB], bf16, tag="Xlo", name=f"Xlo{t}")
                Xhi = xsb_p.tile([128, 2, NM, NB], bf16, tag="Xhi", name=f"Xhi{t}")
                nc.vector.tensor_tensor(Xlo[:], E[:], Fs[:], ADD)
                nc.vector.tensor_tensor(Xhi[:], E[:], Fs[:], SUB)
                state[t] = {"Xlo": Xlo, "Xhi": Xhi}

            def eB(t):
                """intensity: sq (ACT), ri-fold + mode-fold (GPS) -> I_lo, I_hi."""
                st = state[t]
                I = isb_p.tile([128, 2, NB], bf16, tag="I", name=f"I{t}")
                for h, X in enumerate((st["Xlo"], st["Xhi"])):
                    sq = isb_p.tile([128, 2, NM, NB], bf16, tag=f"sq{h}",
                                    name=f"sq{t}_{h}")
                    nc.scalar.activation(sq[:], X[:], SQUARE)
                    s = isb_p.tile([128, NM, NB], bf16, tag=f"s{h}", name=f"s{t}_{h}")
                    nc.gpsimd.tensor_tensor(s[:], sq[:, 0], sq[:, 1], ADD)
                    nc.gpsimd.tensor_tensor(I[:, h], s[:, 0], s[:, 1], ADD)
                st["I"] = I

            def eC(t):
                """corr matmuls -> phi psum bank; duplicate over modes (ACT)."""
                b = t // NQ
                st = state[t]
                I = st["I"]
                # phi bank: [:, q] for q = ri*2 + half
                ph = ps_cor.tile([128, 4, NB], f32, tag="ph", name=f"ph{t}")
                for ri in range(2):
                    for half in range(2):
                        q = ri * 2 + half
                        nc.tensor.matmul(ph[:, q], gslice(gall, b, ri, half, 0),
                                         I[:, 0], start=True, stop=False)
                        nc.tensor.matmul(ph[:, q], gslice(gall, b, ri, half, 1),
                                         I[:, 1], start=False, stop=True)
                # duplicate each phi across the 2 mode slots: DD[:, q] = [phi_q|phi_q]
                DD = dup_p.tile([128, 4, 2, NB], bf16, tag="DD", name=f"DD{t}")
                nc.scalar.copy(DD[:, :, 0], ph[:])
                nc.scalar.copy(DD[:, :, 1], ph[:])
                st["DD"] = DD

            def eD(t):
                """U = j*P*phi .* X  (na = -P*phi_r at q=0,1; nb = -P*phi_i q=2,3).

                Ur = nb*Xr + na*Xi ; Ui = nb*Xi - na*Xr, per half.
                U tiles laid out [128, ri, m, j] like X.
                """
                st = state[t]
                DD = st["DD"]
                U = {}
                for h, X in enumerate((st["Xlo"], st["Xhi"])):
                    na = DD[:, 0 + h]      # [128, 2, NB] duplicated -P*phi_r
                    nb_ = DD[:, 2 + h]     # duplicated -P*phi_i
                    T0 = usb_p.tile([128, 2, NM, NB], bf16, tag=f"T0{h}",
                                    name=f"T0{t}_{h}")
                    T1 = usb_p.tile([128, 2, NM, NB], bf16, tag=f"T1{h}",
                                    name=f"T1{t}_{h}")
                    # T0[:,0]=nb*Xr  T0[:,1]=nb*Xi ; T1[:,0]=na*Xi  T1[:,1]=na*Xr
                    nc.vector.tensor_tensor(T0[:, 0], nb_, X[:, 0], MULT)
                    nc.vector.tensor_tensor(T0[:, 1], nb_, X[:, 1], MULT)
                    nc.vector.tensor_tensor(T1[:, 0], na, X[:, 1], MULT)
                    nc.vector.tensor_tensor(T1[:, 1], na, X[:, 0], MULT)
                    Uh = usb_p.tile([128, 2, NM, NB], bf16, tag=f"U{h}",
                                    name=f"U{t}_{h}")
                    nc.vector.tensor_tensor(Uh[:, 0], T0[:, 0], T1[:, 0], ADD)
                    nc.vector.tensor_tensor(Uh[:, 1], T0[:, 1], T1[:, 1], SUB)
                    U[h] = Uh
                st["U"] = U

            def eE(t):
                """dense IFFT: v_tc = sum_kc IDFT[tc,kc] @ U_kc, out copy + DMA."""
                st = state[t]
                U = st["U"]
                ob = osb_p.tile([128, 2, 2, NM, NB], bf16, tag="ob", name=f"ob{t}")
                for tcn in range(2):
                    vp = ps_ift.tile([128, 2, W2], f32, tag=f"v{tcn}",
                                     name=f"v{t}_{tcn}")
                    mats = [(2 + tcn * 2 + kc) for kc in range(2)]
                    seq_r = []
                    seq_i = []
                    for kc in range(2):
                        mat = mats[kc]
                        Ur, Ui = U[kc][:, 0], U[kc][:, 1]
                        seq_r += [(wslice(wall, mat, 0), Ur),
                                  (wslice(wall, mat, 2), Ui)]
                        seq_i += [(wslice(wall, mat, 0), Ui),
                                  (wslice(wall, mat, 1), Ur)]
                    for ri, seq in enumerate((seq_r, seq_i)):
                        for i, (lhsT, rhs) in enumerate(seq):
                            nc.tensor.matmul(vp[:, ri], lhsT, rhs,
                                             start=(i == 0), stop=(i == 3))
                    nc.scalar.copy(ob[:, tcn], vp[:])
                nc.sync.dma_start(vout_d[t], ob[:])
                del state[t]

            # software pipeline: keep tensor fed two blocks ahead
            eA(0); eB(0); eA(1); eB(1)
            for t in range(NBLK):
                eC(t)
                eD(t)
                if t + 2 < NBLK:
                    eA(t + 2)
                    eB(t + 2)
                eE(t)

    nc.compile()
    return nc


def _run_device(xin_cores, wall_p, gall_p, trace=False):
    global _PROG, LAST_EXEC_NS, LAST_RESULTS
    from concourse.bass_utils import run_bass_kernel_spmd

    if _PROG is None:
        _PROG = _build_program()
    nc = _PROG
    in_maps = []
    for k in range(NCORES):
        in_maps.append({
            "xin": np.ascontiguousarray(xin_cores[k]),
            "wall": wall_p,
            "gall": gall_p,
        })
    kwargs = {}
    if trace:
        kwargs["trace"] = True
    res = run_bass_kernel_spmd(nc, in_maps, list(range(NCORES)), **kwargs)
    LAST_EXEC_NS = res.exec_time_ns
    LAST_RESULTS = res
    return [res.results[k]["vout"] for k in range(NCORES)]


def _emulate_device(xin_cores, wall, gall):
    """Numpy mirror of the device program (f32)."""
    outs = []
    for k in range(NCORES):
        xin = xin_cores[k].astype(np.float32)  # [NBLK, 128, 4, NM, NB]
        vout = np.empty((NBLK, 128, 2, 2, NM, NB), np.float32)
        for t in range(NBLK):
            b = t // NQ
            xer, xei = xin[t, :, 0].reshape(128, W2), xin[t, :, 1].reshape(128, W2)
            xor_, xoi = xin[t, :, 2].reshape(128, W2), xin[t, :, 3].reshape(128, W2)
            Er = wall[0].T @ xer + wall[2].T @ xei
            Ei = wall[0].T @ xei + wall[1].T @ xer
            Fr = wall[3].T @ xor_ + wall[5].T @ xoi
            Fi = wall[3].T @ xoi + wall[4].T @ xor_
            X = {0: (Er + Fr, Ei + Fi), 1: (Er - Fr, Ei - Fi)}
            I = {}
            for h in range(2):
                Xr, Xi = X[h]
                s = (Xr * Xr + Xi * Xi).reshape(128, NM, NB)
                I[h] = s[:, 0] + s[:, 1]
            gq = gall.reshape(B, 2, 2, 2, 128, 128)
            U = {}
            for h in range(2):
                na = gq[b, 0, h, 0].T @ I[0] + gq[b, 0, h, 1].T @ I[1]
                nb_ = gq[b, 1, h, 0].T @ I[0] + gq[b, 1, h, 1].T @ I[1]
                na2 = np.repeat(na[:, None, :], NM, 1).reshape(128, W2)
                nb2 = np.repeat(nb_[:, None, :], NM, 1).reshape(128, W2)
                Xr, Xi = X[h]
                U[h] = (nb2 * Xr + na2 * Xi, nb2 * Xi - na2 * Xr)
            for tcn in range(2):
                acc_r = np.zeros((128, W2), np.float32)
                acc_i = np.zeros((128, W2), np.float32)
                for kc in range(2):
                    mat = 2 + tcn * 2 + kc
                    Ur, Ui = U[kc]
                    acc_r += wall[3 * mat].T @ Ur + wall[3 * mat + 2].T @ Ui
                    acc_i += wall[3 * mat].T @ Ui + wall[3 * mat + 1].T @ Ur
                vout[t, :, tcn, 0] = acc_r.reshape(128, NM, NB)
                vout[t, :, tcn, 1] = acc_i.reshape(128, NM, NB)
        outs.append(vout)
    return outs


def kernel(x_real, x_imag, task_info, h_real, h_imag, _emulate=False, _trace=False):
    x_real = np.asarray(x_real, np.float32)
    x_imag = np.asarray(x_imag, np.float32)
    wall, gall, wall_p, gall_p = _build_const_matrices(h_real, h_imag, task_info)
    F = _frames_view(x_real, x_imag)
    xin_cores = _pack_inputs(F)
    if _emulate:
        vouts = _emulate_device(xin_cores, wall, gall)
    else:
        vouts = _run_device(xin_cores, wall_p, gall_p, trace=_trace)
    vfr = _unpack_outputs(vouts)
    yf = (F + vfr).transpose(0, 1, 2, 4, 3)   # [B, NM, 2, NFFT, FTOT]
    y = _overlap_add(yf)
    y = y[:, :, :, PAD:L - PAD]
    return np.ascontiguousarray(y.transpose(0, 3, 1, 2))


# revision 12
# speedup vs baseline: 1.3829x; 1.0421x over previous
"""Trainium2 Bass kernel for nn_EqStftSnsePBC (STFT -> per-tap nonlinear PBC -> ISTFT).

Strategy (8 NeuronCores, data parallel over STFT frames):
  host:   frame the signal (hop 216, n_fft 256), split each frame into even/odd
          time samples (radix-2 DIT), pack per-block DRAM buffers that mirror
          the SBUF tile layout exactly (one contiguous DMA per block each way).
  device: per block of NB=116 frames (modes packed along columns, 2NB=232):
            E  = W128 @ xe, F = (diag(w) W128) @ xo      (8 matmuls, K=128)
            X_lo = E + F, X_hi = E - F                   (DVE butterfly == PSUM copy)
            I  = sum_modes |X|^2                         (ACT square + DVE folds)
            phi = G @ I  (circulant corr, -P folded)     (8 matmuls)
            U  = j*P*phi .* X                            (DVE elementwise)
            v  = IDFT256 @ U  (dense, 2 row-chunks)      (16 matmuls)
          all bf16 with f32 PSUM accumulation.
  host:   yf = frames + v (exact passthrough of linear term), overlap-add,
          divide by coverage, trim, restack.
"""

import os
import sys

for _p in ("/opt/trn_rl_repo",):
    if os.path.isdir(_p) and _p not in sys.path:
        sys.path.append(_p)

import numpy as np
try:
    import ml_dtypes
    _BF16 = np.dtype(ml_dtypes.bfloat16)
except Exception:
    _BF16 = None

# ---- problem geometry (hardcoded) ----
MTAPS = 41
PAD = MTAPS // 2  # 20
NFFT = 256
HOP = 216
B = 2
NM = 2
L = 999688
STEPS = 4628            # (L - NFFT) // HOP + 1
NCORES = 8
NB = 116                # frames per block
NQ = 5                  # blocks per (core, b)
NH = NQ * NB            # 580 frames per core per b
FTOT = NCORES * NH      # 4640 >= STEPS (12 trailing fake frames, ignored on host)
NBLK = B * NQ           # 10 blocks per core
W2 = 2 * NB             # 232  (modes packed)
W4 = 4 * NB             # 464

_PROG = None
LAST_EXEC_NS = None
LAST_RESULTS = None


def _build_const_matrices(h_real, h_imag, task_info):
    """All lhsT constant matrices, bf16-packed for the device.

    wall [128, 18*128]: radix-2 FFT mats (ME, MF) and dense IDFT blocks,
      each as (Mr, Mi, -Mi) triples of lhsT = M.T.
    gall [128, 16*128]: correlation lhsT blocks per (b, ri, half, kc),
      scaled by -P[b].
    """
    n = np.arange(128)
    W128 = np.exp(-2j * np.pi * np.outer(n, n) / 128.0)        # [k, n]
    w = np.exp(-2j * np.pi * np.arange(128) / 256.0)           # twiddles
    ME = W128                                                   # E = ME @ xe
    MF = w[:, None] * W128                                      # F = MF @ xo
    t = np.arange(NFFT)
    IDFT = np.exp(2j * np.pi * np.outer(t, t) / NFFT) / NFFT    # [t, k]

    mats = [ME, MF]
    for tc in range(2):
        for kc in range(2):
            mats.append(IDFT[tc * 128:(tc + 1) * 128, kc * 128:(kc + 1) * 128])
    # -> 6 complex matrices -> 18 real lhsT blocks (Mr, Mi, -Mi each)
    wall = np.empty((18, 128, 128), np.float32)
    for i, M in enumerate(mats):
        lr, li = M.T.real, M.T.imag    # lhsT[n, k] = M[k, n]
        wall[3 * i + 0] = lr
        wall[3 * i + 1] = li
        wall[3 * i + 2] = -li

    # G[p', m] circulant correlation matrix (same as reference semantics)
    def toep(h):
        G = np.zeros((NFFT, NFFT), np.float64)
        for p in range(NFFT + 2 * PAD):
            pp = (p - PAD) % NFFT
            lo, hi = max(0, p - (MTAPS - 1)), min(NFFT - 1, p)
            if lo <= hi:
                ms = np.arange(lo, hi + 1)
                G[pp, ms] += h[p - ms]
        return G

    Gr = toep(np.asarray(h_real, np.float64))
    Gi = toep(np.asarray(h_imag, np.float64))
    P = 10.0 ** (np.asarray(task_info, np.float64)[:, 0] / 10.0) / NM
    # toep's G is [source_freq, output_freq]: phi = G.T @ I, so the lhsT
    # block for output-half `half`, input-chunk `kc` is G[kc rows, half cols].
    gall = np.empty((B, 2, 2, 2, 128, 128), np.float32)  # [b, ri, half, kc, n, k]
    for b in range(B):
        for ri, G in enumerate((Gr, Gi)):
            for half in range(2):
                for kc in range(2):
                    blk = G[kc * 128:(kc + 1) * 128, half * 128:(half + 1) * 128]
                    gall[b, ri, half, kc] = (-P[b] * blk)
    wall_p = np.ascontiguousarray(
        wall.transpose(1, 0, 2).reshape(128, 18 * 128)).astype(_BF16)
    gall_p = np.ascontiguousarray(
        gall.reshape(16, 128, 128).transpose(1, 0, 2).reshape(128, 16 * 128)
    ).astype(_BF16)
    return wall, gall, wall_p, gall_p


def _frames_view(x_real, x_imag):
    """-> F [B, NM, 2, FTOT, NFFT] float32 frames (zero-padded past L)."""
    need = HOP * (FTOT - 1) + NFFT
    F = np.empty((B, NM, 2, FTOT, NFFT), np.float32)
    for ri, x in enumerate((x_real, x_imag)):
        xt = np.ascontiguousarray(np.asarray(x, np.float32).transpose(0, 2, 1))
        xp = np.zeros((B, NM, need), np.float32)
        xp[:, :, :L] = xt
        sw = np.lib.stride_tricks.as_strided(
            xp, shape=(B, NM, FTOT, NFFT),
            strides=(xp.strides[0], xp.strides[1], HOP * 4, 4))
        F[:, :, ri] = sw
    return F


def _pack_inputs(F):
    """F [B,NM,2,FTOT,256] -> per-core xin [NCORES][NBLK,128,4,NM,NB] bf16.

    s index = eo*2 + ri: 0=(even,re) 1=(even,im) 2=(odd,re) 3=(odd,im).
    """
    xe = F[..., 0::2]   # [B, NM, 2, FTOT, 128]
    xo = F[..., 1::2]
    arr = np.stack([xe, xo], axis=0)  # [eo, B, NM, ri, FTOT, 128]
    r1 = arr.reshape(2, B, NM, 2, NCORES, NQ, NB, 128)
    # -> [k, b, q, n, eo, ri, m, j]
    out = r1.transpose(4, 1, 5, 7, 0, 3, 2, 6)
    out = np.ascontiguousarray(out).astype(_BF16)
    return out.reshape(NCORES, NBLK, 128, 4, NM, NB)


def _unpack_outputs(vouts):
    """vouts [NCORES][NBLK,128,2,2,NM,NB] -> v frames [B,NM,2,FTOT,256] f32."""
    va = np.stack([v.astype(np.float32) for v in vouts], axis=0)
    # dims [k, b, q, n, tc, ri, m, j] -> [b, m, ri, k, q, j, tc, n]
    va = va.reshape(NCORES, B, NQ, 128, 2, 2, NM, NB)
    vfr = va.transpose(1, 6, 5, 0, 2, 7, 4, 3).reshape(B, NM, 2, FTOT, NFFT)
    return vfr


def _overlap_add(yf):
    """yf [B, NM, 2, NFFT, FTOT] -> y [B, NM, 2, L] (OLA / coverage)."""
    y = np.zeros((B, NM, 2, STEPS, HOP), np.float32)
    body = yf[:, :, :, :HOP, :STEPS].transpose(0, 1, 2, 4, 3)
    y[:] = body
    tail = yf[:, :, :, HOP:, :STEPS - 1].transpose(0, 1, 2, 4, 3)
    y[:, :, :, 1:, :NFFT - HOP] += tail
    y = y.reshape(B, NM, 2, STEPS * HOP)
    yfull = np.empty((B, NM, 2, L), np.float32)
    yfull[:, :, :, :STEPS * HOP] = y
    yfull[:, :, :, STEPS * HOP:] = yf[:, :, :, HOP:HOP + (L - STEPS * HOP), STEPS - 1]
    t = np.arange(L)
    wsum = np.ones(L, np.float32)
    wsum[(t >= HOP) & (t < STEPS * HOP) & (t % HOP < NFFT - HOP)] = 2.0
    yfull /= wsum
    return yfull


def _build_program():
    import concourse.bass as bass
    import concourse.tile as tile
    from concourse import bacc, mybir
    from contextlib import ExitStack

    f32 = mybir.dt.float32
    bf16 = mybir.dt.bfloat16
    MULT = mybir.AluOpType.mult
    ADD = mybir.AluOpType.add
    SUB = mybir.AluOpType.subtract
    SQUARE = mybir.ActivationFunctionType.Square

    nc = bacc.Bacc(None, target_bir_lowering=False, debug=False)
    xin_d = nc.dram_tensor("xin", [NBLK, 128, 8, NB], bf16,
                           kind="ExternalInput").ap()
    wall_d = nc.dram_tensor("wall", [128, 18 * 128], bf16,
                            kind="ExternalInput").ap()
    gall_d = nc.dram_tensor("gall", [128, 16 * 128], bf16,
                            kind="ExternalInput").ap()
    vout_d = nc.dram_tensor("vout", [NBLK, 128, 2, 2 * W2], bf16,
                            kind="ExternalOutput").ap()

    # wall block index: (mat, part) mat in [ME, MF, I00, I01, I10, I11],
    # part in [r, i, negi]
    def wslice(wall_sb, mat, part):
        off = (mat * 3 + part) * 128
        return wall_sb[:, off:off + 128]

    def gslice(gall_sb, b, ri, half, kc):
        off = (((b * 2 + ri) * 2 + half) * 2 + kc) * 128
        return gall_sb[:, off:off + 128]

    with tile.TileContext(nc) as tc:
        with ExitStack() as ctx:
            consts = ctx.enter_context(tc.tile_pool(name="consts", bufs=1))
            xin_p = ctx.enter_context(tc.tile_pool(name="xin", bufs=4))
            xsb_p = ctx.enter_context(tc.tile_pool(name="xsb", bufs=3))
            isb_p = ctx.enter_context(tc.tile_pool(name="isb", bufs=3))
            dup_p = ctx.enter_context(tc.tile_pool(name="dup", bufs=2))
            usb_p = ctx.enter_context(tc.tile_pool(name="usb", bufs=2))
            osb_p = ctx.enter_context(tc.tile_pool(name="osb", bufs=2))
            ps_fft = ctx.enter_context(tc.tile_pool(name="psf", bufs=2, space="PSUM"))
            ps_cor = ctx.enter_context(tc.tile_pool(name="psc", bufs=2, space="PSUM"))
            ps_ift = ctx.enter_context(tc.tile_pool(name="psv", bufs=1, space="PSUM"))

            # split const loads: FFT matrices (first 6 blocks of wall) arrive
            # first on their own queue so block 0 can start ASAP.
            wall = consts.tile([128, 18 * 128], bf16, tag="wall")
            nc.sync.dma_start(wall[:, :6 * 128], wall_d[:, :6 * 128])
            nc.scalar.dma_start(wall[:, 6 * 128:], wall_d[:, 6 * 128:])
            gall = consts.tile([128, 16 * 128], bf16, tag="gall")
            nc.scalar.dma_start(gall[:], gall_d[:])

            state = {}

            def eDMA(t):
                xin = xin_p.tile([128, 8, NB], bf16, tag="xin", name=f"xin{t}")
                nc.sync.dma_start(xin[:], xin_d[t])
                state[t] = {"xin": xin}

            def eA(t):
                """FFT matmuls + butterfly combine -> X in SBUF (flat [128,464])."""
                xin = state[t]["xin"]
                E = ps_fft.tile([128, 4, NB], f32, tag="E", name=f"E{t}")
                Fp = ps_fft.tile([128, 4, NB], f32, tag="F", name=f"F{t}")
                for (ps, mat, ur, ui) in ((E, 0, xin[:, 0:2], xin[:, 2:4]),
                                          (Fp, 1, xin[:, 4:6], xin[:, 6:8])):
                    mr = wslice(wall, mat, 0)
                    mi = wslice(wall, mat, 1)
                    mni = wslice(wall, mat, 2)
                    nc.tensor.matmul(ps[:, 0:2], mr, ur, start=True, stop=False)
                    nc.tensor.matmul(ps[:, 0:2], mni, ui, start=False, stop=True)
                    nc.tensor.matmul(ps[:, 2:4], mr, ui, start=True, stop=False)
                    nc.tensor.matmul(ps[:, 2:4], mi, ur, start=False, stop=True)
                # butterfly: X_lo = E + F, X_hi = E - F  (to SBUF bf16).
                # TT can read only one PSUM operand, so stage F through SBUF.
                Fs = xsb_p.tile([128, 4, NB], bf16, tag="Fs", name=f"Fs{t}")
                nc.scalar.copy(Fs[:], Fp[:])
                Xlo = xsb_p.tile([128, 4, NB], bf16, tag="Xlo", name=f"Xlo{t}")
                Xhi = xsb_p.tile([128, 4, NB], bf16, tag="Xhi", name=f"Xhi{t}")
                nc.vector.tensor_tensor(Xlo[:], E[:], Fs[:], ADD)
                nc.vector.tensor_tensor(Xhi[:], E[:], Fs[:], SUB)
                state[t].update({"Xlo": Xlo, "Xhi": Xhi})

            def eB(t):
                """intensity: sq (ACT), ri-fold (DVE) + mode-fold (GPS)."""
                st = state[t]
                I = isb_p.tile([128, 2, NB], bf16, tag="I", name=f"I{t}")
                for h, X in enumerate((st["Xlo"], st["Xhi"])):
                    sq = isb_p.tile([128, 4, NB], bf16, tag=f"sq{h}",
                                    name=f"sq{t}_{h}")
                    nc.scalar.activation(sq[:], X[:], SQUARE)
                    s = isb_p.tile([128, 2, NB], bf16, tag=f"s{h}", name=f"s{t}_{h}")
                    nc.gpsimd.tensor_tensor(s[:], sq[:, 0:2], sq[:, 2:4], ADD)
                    nc.gpsimd.tensor_tensor(I[:, h], s[:, 0], s[:, 1], ADD)
                st["I"] = I

            def eC(t):
                """corr matmuls -> phi psum bank; single bf16 copy (no dup)."""
                b = t // NQ
                st = state[t]
                I = st["I"]
                # phi bank [128, 4, NB]: q = ri*2 + half
                ph = ps_cor.tile([128, 4, 1, NB], f32, tag="ph", name=f"ph{t}")
                for ri in range(2):
                    for half in range(2):
                        q = ri * 2 + half
                        nc.tensor.matmul(ph[:, q], gslice(gall, b, ri, half, 0),
                                         I[:, 0], start=True, stop=False)
                        nc.tensor.matmul(ph[:, q], gslice(gall, b, ri, half, 1),
                                         I[:, 1], start=False, stop=True)
                phs = dup_p.tile([128, 4, 1, NB], bf16, tag="phs", name=f"phs{t}")
                nc.scalar.copy(phs[:], ph[:])
                st["phs"] = phs

            def eD(t):
                """U = j*P*phi .* X via broadcast reads of phi.

                X_h [128,4,NB] = (ri*m, j); T0 = nb*X, T1 = na*X (phi bcast
                over ri,m); Ur = T0[r]+T1[i], Ui = T0[i]-T1[r].
                """
                st = state[t]
                phs = st["phs"]
                U = {}
                for h, X in enumerate((st["Xlo"], st["Xhi"])):
                    naB = phs[:, 0 + h].broadcast_to([128, 4, NB])
                    nbB = phs[:, 2 + h].broadcast_to([128, 4, NB])
                    T0 = usb_p.tile([128, 4, NB], bf16, tag=f"T0{h}",
                                    name=f"T0{t}_{h}")
                    T1 = usb_p.tile([128, 4, NB], bf16, tag=f"T1{h}",
                                    name=f"T1{t}_{h}")
                    nc.vector.tensor_tensor(T0[:], X[:], nbB, MULT)
                    nc.vector.tensor_tensor(T1[:], X[:], naB, MULT)
                    Uh = usb_p.tile([128, 4, NB], bf16, tag=f"U{h}",
                                    name=f"U{t}_{h}")
                    nc.vector.tensor_tensor(Uh[:, 0:2], T0[:, 0:2], T1[:, 2:4], ADD)
                    nc.vector.tensor_tensor(Uh[:, 2:4], T0[:, 2:4], T1[:, 0:2], SUB)
                    U[h] = Uh
                st["U"] = U

            def eE(t):
                """dense IFFT: v_tc = sum_kc IDFT[tc,kc] @ U_kc, out copy + DMA."""
                st = state[t]
                U = st["U"]
                # single 2-bank psum [128, 2, 512]; used cols [0:464] per tc
                vp = ps_ift.tile([128, 2, 512], f32, tag="vp", name=f"vp{t}")
                for tcn in range(2):
                    mats = [(2 + tcn * 2 + kc) for kc in range(2)]
                    seq_r = []
                    seq_i = []
                    for kc in range(2):
                        mat = mats[kc]
                        Ur, Ui = U[kc][:, 0:2], U[kc][:, 2:4]
                        seq_r += [(wslice(wall, mat, 0), Ur),
                                  (wslice(wall, mat, 2), Ui)]
                        seq_i += [(wslice(wall, mat, 0), Ui),
                                  (wslice(wall, mat, 1), Ur)]
                    for ri, seq in enumerate((seq_r, seq_i)):
                        for i, (lhsT, rhs) in enumerate(seq):
                            nc.tensor.matmul(vp[:, tcn, ri * W2:(ri + 1) * W2],
                                             lhsT, rhs,
                                             start=(i == 0), stop=(i == 3))
                ob = osb_p.tile([128, 2, 2 * W2], bf16, tag="ob", name=f"ob{t}")
                nc.scalar.copy(ob[:], vp[:, :, :2 * W2])
                nc.gpsimd.dma_start(vout_d[t], ob[:])
                del state[t]

            # software pipeline: keep tensor fed two blocks ahead
            eDMA(0); eDMA(1); eDMA(2)
            eA(0); eB(0); eA(1); eB(1)
            for t in range(NBLK):
                eC(t)
                eD(t)
                if t + 3 < NBLK:
                    eDMA(t + 3)
                if t + 2 < NBLK:
                    eA(t + 2)
                    eB(t + 2)
                eE(t)

    nc.compile()
    return nc


def _run_device(xin_cores, wall_p, gall_p, trace=False):
    global _PROG, LAST_EXEC_NS, LAST_RESULTS
    from concourse.bass_utils import run_bass_kernel_spmd

    if _PROG is None:
        _PROG = _build_program()
    nc = _PROG
    in_maps = []
    for k in range(NCORES):
        in_maps.append({
            "xin": np.ascontiguousarray(xin_cores[k]).reshape(NBLK, 128, 8, NB),
            "wall": wall_p,
            "gall": gall_p,
        })
    kwargs = {}
    if trace:
        kwargs["trace"] = True
    res = run_bass_kernel_spmd(nc, in_maps, list(range(NCORES)), **kwargs)
    LAST_EXEC_NS = res.exec_time_ns
    LAST_RESULTS = res
    return [res.results[k]["vout"] for k in range(NCORES)]


def _emulate_device(xin_cores, wall, gall):
    """Numpy mirror of the device program (f32)."""
    outs = []
    for k in range(NCORES):
        xin = xin_cores[k].astype(np.float32)  # [NBLK, 128, 4, NM, NB]
        vout = np.empty((NBLK, 128, 2, 2, NM, NB), np.float32)
        for t in range(NBLK):
            b = t // NQ
            xer, xei = xin[t, :, 0].reshape(128, W2), xin[t, :, 1].reshape(128, W2)
            xor_, xoi = xin[t, :, 2].reshape(128, W2), xin[t, :, 3].reshape(128, W2)
            Er = wall[0].T @ xer + wall[2].T @ xei
            Ei = wall[0].T @ xei + wall[1].T @ xer
            Fr = wall[3].T @ xor_ + wall[5].T @ xoi
            Fi = wall[3].T @ xoi + wall[4].T @ xor_
            X = {0: (Er + Fr, Ei + Fi), 1: (Er - Fr, Ei - Fi)}
            I = {}
            for h in range(2):
                Xr, Xi = X[h]
                s = (Xr * Xr + Xi * Xi).reshape(128, NM, NB)
                I[h] = s[:, 0] + s[:, 1]
            gq = gall.reshape(B, 2, 2, 2, 128, 128)
            U = {}
            for h in range(2):
                na = gq[b, 0, h, 0].T @ I[0] + gq[b, 0, h, 1].T @ I[1]
                nb_ = gq[b, 1, h, 0].T @ I[0] + gq[b, 1, h, 1].T @ I[1]
                na2 = np.repeat(na[:, None, :], NM, 1).reshape(128, W2)
                nb2 = np.repeat(nb_[:, None, :], NM, 1).reshape(128, W2)
                Xr, Xi = X[h]
                U[h] = (nb2 * Xr + na2 * Xi, nb2 * Xi - na2 * Xr)
            for tcn in range(2):
                acc_r = np.zeros((128, W2), np.float32)
                acc_i = np.zeros((128, W2), np.float32)
                for kc in range(2):
                    mat = 2 + tcn * 2 + kc
                    Ur, Ui = U[kc]
                    acc_r += wall[3 * mat].T @ Ur + wall[3 * mat + 2].T @ Ui
                    acc_i += wall[3 * mat].T @ Ui + wall[3 * mat + 1].T @ Ur
                vout[t, :, tcn, 0] = acc_r.reshape(128, NM, NB)
                vout[t, :, tcn, 1] = acc_i.reshape(128, NM, NB)
        outs.append(vout)
    return outs


def kernel(x_real, x_imag, task_info, h_real, h_imag, _emulate=False, _trace=False):
    x_real = np.asarray(x_real, np.float32)
    x_imag = np.asarray(x_imag, np.float32)
    wall, gall, wall_p, gall_p = _build_const_matrices(h_real, h_imag, task_info)
    F = _frames_view(x_real, x_imag)
    xin_cores = _pack_inputs(F)
    if _emulate:
        vouts = _emulate_device(xin_cores, wall, gall)
    else:
        vouts = _run_device(xin_cores, wall_p, gall_p, trace=_trace)
    vfr = _unpack_outputs(vouts)
    yf = (F + vfr).transpose(0, 1, 2, 4, 3)   # [B, NM, 2, NFFT, FTOT]
    y = _overlap_add(yf)
    y = y[:, :, :, PAD:L - PAD]
    return np.ascontiguousarray(y.transpose(0, 3, 1, 2))
